# revision 1
# baseline (speedup 1.0000x reference)
"""KNN classifier (B=4096, P=50000, C=100, K=5) on 8 Trainium2 NeuronCores.

Algorithm (per core, 512 rows, data-parallel over batch):
  pass 1  stream x in [128, STRIP] tiles, per-256-col chunk mins (DVE tensor_reduce)
  phase 2 top-5 chunks per row by (min, idx) via max8 + max_index; sort chunk ids
  phase 3 indirect-DMA gather the 5 winning 256-col chunks per row (ascending col)
  phase 4 max8 + max_index over the gathered 1280 elems -> exact top-5 (value, col)
          with jax.lax.top_k tie semantics (ties -> lowest column index)
  phase 5 indirect-DMA gather one-hot label rows, sum, argmax (lowest class on tie)

Correctness of phase 2/3: the 5 smallest elements of a row always lie inside the
5 chunks with lexicographically smallest (chunk-min, chunk-idx): if an element e
of rank <=5 sat in a chunk displaced by 5 lex-smaller chunks, each of those
chunks would contain an element ranked before e, contradicting rank(e) <= 5.
"""

import numpy as np

B, P, C, K = 4096, 50000, 100, 5
N_CORES = 8
ROWS_PER_CORE = B // N_CORES  # 512
CHUNK = 256
STRIP = 3584                  # 14 chunks per strip
PPAD = 50176                  # 14 strips per row
FPAD = np.float32(3.0e38)     # finite "+inf" pad (x values are uniform [0,1))
BIGF = 3.0e7

SORT5_NET = [(0, 1), (3, 4), (2, 4), (2, 3), (1, 4), (0, 3), (0, 2), (1, 3), (1, 2)]


def build_knn(tc, y_ap, x_ap, oh_ap, *, rows, ppad, chunk, strip, ncls):
    """Emit the per-core KNN program. x_ap: [rows, ppad] f32 (padded with FPAD),
    oh_ap: [nreal, ncls] f32 one-hot, y_ap: [rows, 1] int32 out."""
    import concourse.mybir as mybir
    from concourse import bass

    nc = tc.nc
    A = mybir.AluOpType
    f32 = mybir.dt.float32
    i32 = mybir.dt.int32
    u32 = mybir.dt.uint32
    COPY = mybir.ActivationFunctionType.Copy
    X = mybir.AxisListType.X

    nblocks = rows // 128
    nchunks = ppad // chunk
    nstrips = ppad // strip
    cps = strip // chunk
    KC = K * chunk

    xv = x_ap.rearrange("r (c v) -> (r c) v", v=chunk)  # [rows*nchunks, chunk] DRAM view

    with (
        tc.tile_pool(name="const", bufs=1) as cpool,
        tc.tile_pool(name="stream", bufs=4) as spool,
        tc.tile_pool(name="work", bufs=2) as wpool,
    ):
        # constants: per-partition row id and class iota, both as f32
        rg_u = cpool.tile([128, 1], u32)
        nc.gpsimd.iota(rg_u[:], pattern=[[0, 1]], base=0, channel_multiplier=1)
        rgf = cpool.tile([128, 1], f32)
        nc.vector.tensor_copy(rgf[:], rg_u[:])
        iotaC_u = cpool.tile([128, ncls], u32)
        nc.gpsimd.iota(iotaC_u[:], pattern=[[1, ncls]], base=0, channel_multiplier=0)
        iotaC = cpool.tile([128, ncls], f32)
        nc.vector.tensor_copy(iotaC[:], iotaC_u[:])

        for b in range(nblocks):
            # ---- pass 1: chunk mins ----
            M = wpool.tile([128, nchunks], f32, tag="M")
            for s in range(nstrips):
                xs = spool.tile([128, strip], f32, tag="xs")
                nc.sync.dma_start(
                    out=xs[:], in_=x_ap[b * 128 : (b + 1) * 128, s * strip : (s + 1) * strip]
                )
                nc.vector.tensor_reduce(
                    out=M[:, s * cps : (s + 1) * cps],
                    in_=xs[:].rearrange("p (c v) -> p c v", v=chunk),
                    axis=X,
                    op=A.min,
                )
            # ---- phase 2: top-5 chunks by (min, idx), sorted by idx ----
            Mn = wpool.tile([128, nchunks], f32, tag="Mn")
            nc.scalar.activation(Mn[:], M[:], COPY, scale=-1.0)
            c8 = wpool.tile([128, 8], f32, tag="c8")
            nc.vector.max(out=c8[:], in_=Mn[:])
            ci8 = wpool.tile([128, 8], u32, tag="ci8")
            nc.vector.max_index(out=ci8[:], in_max=c8[:], in_values=Mn[:])
            cif8 = wpool.tile([128, 8], f32, tag="cif8")
            nc.vector.tensor_copy(cif8[:], ci8[:])
            cif = wpool.tile([128, K], f32, tag="cif")
            nc.vector.tensor_copy(cif[:], cif8[:, 0:K])
            tmin = wpool.tile([128, 1], f32, tag="tmin")
            tmax = wpool.tile([128, 1], f32, tag="tmax")
            for i, j in SORT5_NET:
                nc.vector.tensor_tensor(out=tmin[:], in0=cif[:, i : i + 1], in1=cif[:, j : j + 1], op=A.min)
                nc.vector.tensor_tensor(out=tmax[:], in0=cif[:, i : i + 1], in1=cif[:, j : j + 1], op=A.max)
                nc.vector.tensor_copy(cif[:, i : i + 1], tmin[:])
                nc.vector.tensor_copy(cif[:, j : j + 1], tmax[:])
            # flat chunk index = (row + b*128) * nchunks + chunk_id
            rbn = wpool.tile([128, 1], f32, tag="rbn")
            nc.vector.tensor_scalar(rbn[:], rgf[:], float(b * 128), float(nchunks), op0=A.add, op1=A.mult)
            cidxf = wpool.tile([128, K], f32, tag="cidxf")
            nc.vector.tensor_scalar(cidxf[:], cif[:], rbn[:, 0:1], None, op0=A.add)
            cidx = wpool.tile([128, K], i32, tag="cidx")
            nc.vector.tensor_copy(cidx[:], cidxf[:])
            # ---- phase 3: gather winning chunks ----
            G = wpool.tile([128, KC], f32, tag="G")
            for t in range(K):
                nc.gpsimd.indirect_dma_start(
                    out=G[:, t * chunk : (t + 1) * chunk],
                    out_offset=None,
                    in_=xv[:, :],
                    in_offset=bass.IndirectOffsetOnAxis(ap=cidx[:, t : t + 1], axis=0),
                )
            # ---- phase 4: exact top-5 (value, col) ----
            Gn = wpool.tile([128, KC], f32, tag="Gn")
            nc.scalar.activation(Gn[:], G[:], COPY, scale=-1.0)
            g8 = wpool.tile([128, 8], f32, tag="g8")
            nc.vector.max(out=g8[:], in_=Gn[:])
            p8 = wpool.tile([128, 8], u32, tag="p8")
            nc.vector.max_index(out=p8[:], in_max=g8[:], in_values=Gn[:])
            pf = wpool.tile([128, 8], f32, tag="pf")
            nc.vector.tensor_copy(pf[:], p8[:])
            # tf = floor(pf / chunk) via threshold sums; uf = pf - chunk*tf
            tf = wpool.tile([128, 8], f32, tag="tf")
            ge = wpool.tile([128, 8], f32, tag="ge")
            nc.vector.tensor_scalar(tf[:], pf[:], float(chunk), None, op0=A.is_ge)
            for t in range(2, K):
                nc.vector.tensor_scalar(ge[:], pf[:], float(t * chunk), None, op0=A.is_ge)
                nc.vector.tensor_tensor(out=tf[:], in0=tf[:], in1=ge[:], op=A.add)
            uf = wpool.tile([128, 8], f32, tag="uf")
            nc.vector.tensor_scalar(ge[:], tf[:], float(chunk), None, op0=A.mult)
            nc.vector.tensor_tensor(out=uf[:], in0=pf[:], in1=ge[:], op=A.subtract)
            # map slot t -> sorted chunk id, then global col = chunk_id*chunk + uf
            acc = wpool.tile([128, 8], f32, tag="acc")
            eqt = wpool.tile([128, 8], f32, tag="eqt")
            nc.vector.memset(acc[:], 0.0)
            for t in range(K):
                nc.vector.tensor_scalar(eqt[:], tf[:], float(t), None, op0=A.is_equal)
                nc.vector.tensor_scalar(eqt[:], eqt[:], cif[:, t : t + 1], None, op0=A.mult)
                nc.vector.tensor_tensor(out=acc[:], in0=acc[:], in1=eqt[:], op=A.add)
            colg = wpool.tile([128, 8], f32, tag="colg")
            nc.vector.tensor_scalar(colg[:], acc[:], float(chunk), None, op0=A.mult)
            nc.vector.tensor_tensor(out=colg[:], in0=colg[:], in1=uf[:], op=A.add)
            wcol = wpool.tile([128, K], i32, tag="wcol")
            nc.vector.tensor_copy(wcol[:], colg[:, 0:K])
            # ---- phase 5: labels, vote, argmax (lowest class on tie) ----
            LB = wpool.tile([128, K * ncls], f32, tag="LB")
            for t in range(K):
                nc.gpsimd.indirect_dma_start(
                    out=LB[:, t * ncls : (t + 1) * ncls],
                    out_offset=None,
                    in_=oh_ap[:, :],
                    in_offset=bass.IndirectOffsetOnAxis(ap=wcol[:, t : t + 1], axis=0),
                )
            counts = wpool.tile([128, ncls], f32, tag="counts")
            nc.vector.tensor_reduce(
                out=counts[:], in_=LB[:].rearrange("p (t c) -> p c t", t=K), axis=X, op=A.add
            )
            mx = wpool.tile([128, 1], f32, tag="mx")
            nc.vector.tensor_reduce(out=mx[:], in_=counts[:], axis=X, op=A.max)
            eqc = wpool.tile([128, ncls], f32, tag="eqc")
            nc.vector.tensor_scalar(eqc[:], counts[:], mx[:, 0:1], None, op0=A.is_equal)
            sc2 = wpool.tile([128, ncls], f32, tag="sc2")
            nc.vector.tensor_scalar(sc2[:], eqc[:], -BIGF, BIGF, op0=A.mult, op1=A.add)
            sc1 = wpool.tile([128, ncls], f32, tag="sc1")
            nc.vector.tensor_tensor(out=sc1[:], in0=eqc[:], in1=iotaC[:], op=A.mult)
            nc.vector.tensor_tensor(out=sc1[:], in0=sc1[:], in1=sc2[:], op=A.add)
            yf = wpool.tile([128, 1], f32, tag="yf")
            nc.vector.tensor_reduce(out=yf[:], in_=sc1[:], axis=X, op=A.min)
            yi = wpool.tile([128, 1], i32, tag="yi")
            nc.vector.tensor_copy(yi[:], yf[:])
            nc.sync.dma_start(out=y_ap[b * 128 : (b + 1) * 128, :], in_=yi[:])


_NC_CACHE = {}


def _build_full_nc():
    """Build + compile the per-core program for the real problem shape."""
    if "nc" in _NC_CACHE:
        return _NC_CACHE["nc"]
    import concourse.bacc as bacc
    import concourse.mybir as mybir
    from concourse import tile

    nc = bacc.Bacc("TRN2", target_bir_lowering=False, debug=False, num_devices=N_CORES)
    x_d = nc.dram_tensor("x", [ROWS_PER_CORE, PPAD], mybir.dt.float32, kind="ExternalInput")
    oh_d = nc.dram_tensor("oh", [P, C], mybir.dt.float32, kind="ExternalInput")
    y_d = nc.dram_tensor("y", [ROWS_PER_CORE, 1], mybir.dt.int32, kind="ExternalOutput")
    with tile.TileContext(nc) as tc:
        build_knn(
            tc, y_d.ap(), x_d.ap(), oh_d.ap(),
            rows=ROWS_PER_CORE, ppad=PPAD, chunk=CHUNK, strip=STRIP, ncls=C,
        )
    nc.compile()
    _NC_CACHE["nc"] = nc
    return nc


def _shard_inputs(x, oh):
    x = np.ascontiguousarray(np.asarray(x, dtype=np.float32))
    oh = np.ascontiguousarray(np.asarray(oh, dtype=np.float32))
    assert x.shape == (B, P) and oh.shape == (P, C)
    xpad = np.full((B, PPAD), FPAD, dtype=np.float32)
    xpad[:, :P] = x
    in_maps = []
    for c in range(N_CORES):
        in_maps.append(
            {"x": xpad[c * ROWS_PER_CORE : (c + 1) * ROWS_PER_CORE], "oh": oh}
        )
    return in_maps


def run(x, oh, trace=False):
    from concourse.bass_utils import run_bass_kernel_spmd

    nc = _build_full_nc()
    in_maps = _shard_inputs(x, oh)
    res = run_bass_kernel_spmd(nc, in_maps, core_ids=list(range(N_CORES)), trace=trace)
    y = np.concatenate([res.results[c]["y"].reshape(ROWS_PER_CORE) for c in range(N_CORES)])
    return y.astype(np.int32), res


def kernel(**inputs):
    x = inputs["x"]
    oh = inputs["oh_prototype_labels"]
    k = int(np.asarray(inputs["k"]))
    assert k == K, f"kernel compiled for k={K}, got {k}"
    y, _ = run(x, oh, trace=False)
    return y
